# revision 1
# baseline (speedup 1.0000x reference)
"""Compact Bilinear Pooling on 8 Trainium2 NeuronCores.

Math: for each sample b, Output[b] = sum over pixels p of
  countsketch(x1_p) (circular-conv) countsketch(x2_p)
which, because the sum over pixels commutes with the bilinear pair
products, equals a scatter-reduce of the per-sample gram matrix
  G_b[c1, c2] = sum_p x1[b,p,c1] * x2[b,p,c2]
into buckets d = (h1[c1] + h2[c2]) mod 8192 with signs s1[c1]*s2[c2].

Device plan (two launches, both index-independent programs):
  Phase 1 (batch-sharded, 4 samples/core): G_b = X1_b^T @ X2_b on the
    tensor engine -> DRAM.
  Host: zero-FLOP reshard. The pair->bucket map is compile-time data
    (tiny int index vectors); pairs are laid out into a padded
    bucket-major table, split into positive-sign and negative-sign
    tables (so no sign arithmetic is ever needed anywhere).
  Phase 2 (bucket-sharded, 1024 buckets/core): segmented sums via
    vector-engine reduction; out = pos_sum - neg_sum.
"""

import numpy as np

import concourse.bass as bass
import concourse.bacc as bacc
import concourse.mybir as mybir
from concourse.tile import TileContext
from concourse import bass_utils

B, C, HW, D = 32, 512, 196, 8192
NCORES = 8
BPC = B // NCORES          # samples per core in phase 1
DPC = D // NCORES          # buckets per core in phase 2
F32 = mybir.dt.float32
F32R = mybir.dt.float32r   # TF32-like PE mode: 1 cycle/row vs 4 for fp32
BF16 = mybir.dt.bfloat16
G_DT = BF16                # gram matrix precision on the wire

_cache = {}
_last_runs = []  # (nc, in_maps) of the most recent kernel() call, for profiling


def _build_phase1():
    """Per core: x1,x2 [BPC, 196, 512] f32 -> g [BPC, 512, 512] f32."""
    nc = bacc.Bacc("TRN2", target_bir_lowering=False, debug=False,
                   num_devices=NCORES)
    x1 = nc.dram_tensor("x1", [BPC, HW, C], F32R, kind="ExternalInput").ap()
    x2 = nc.dram_tensor("x2", [BPC, HW, C], F32R, kind="ExternalInput").ap()
    g = nc.dram_tensor("g", [BPC, C, C], G_DT, kind="ExternalOutput").ap()

    KA, KB = 128, HW - 128  # pixel (contraction) dim split

    with TileContext(nc) as tc:
        with (
            tc.tile_pool(name="xp", bufs=3) as xp,
            tc.tile_pool(name="gp", bufs=4) as gp,
            tc.tile_pool(name="ps", bufs=8, space="PSUM") as ps,
        ):
            for b in range(BPC):
                x1a = xp.tile([KA, C], F32R, tag="x1a")
                x1b = xp.tile([KB, C], F32R, tag="x1b")
                x2a = xp.tile([KA, C], F32R, tag="x2a")
                x2b = xp.tile([KB, C], F32R, tag="x2b")
                nc.sync.dma_start(x1a[:], x1[b, 0:KA, :])
                nc.sync.dma_start(x1b[:], x1[b, KA:HW, :])
                nc.sync.dma_start(x2a[:], x2[b, 0:KA, :])
                nc.sync.dma_start(x2b[:], x2[b, KA:HW, :])
                for m in range(C // 128):
                    pt = ps.tile([128, C], F32)
                    nc.tensor.matmul(pt[:], x1a[:, m * 128:(m + 1) * 128],
                                     x2a[:], start=True, stop=False)
                    nc.tensor.matmul(pt[:], x1b[:, m * 128:(m + 1) * 128],
                                     x2b[:], start=False, stop=True)
                    gt = gp.tile([128, C], G_DT)
                    nc.vector.tensor_copy(gt[:], pt[:])
                    nc.sync.dma_start(g[b, m * 128:(m + 1) * 128, :], gt[:])
    nc.compile()
    return nc


def _build_phase2(cap):
    """Per core: t [DPC, B, cap] bf16 (bucket-major padded pair values),
    mask [DPC, cap] bf16 (+-1 per slot, shared across samples) ->
    out [DPC, B] f32 = sum over slots of t * mask."""
    nc = bacc.Bacc("TRN2", target_bir_lowering=False, debug=False,
                   num_devices=NCORES)
    NJ = DPC // 128
    t = nc.dram_tensor("t", [DPC, B, cap], G_DT, kind="ExternalInput").ap()
    # partition-major output; host transposes it back (layout only)
    out = nc.dram_tensor("out", [128, NJ, B], F32, kind="ExternalOutput").ap()

    with TileContext(nc) as tc:
        with (
            tc.tile_pool(name="tb", bufs=NJ + 1) as tb,
            tc.tile_pool(name="ob", bufs=1) as ob,
        ):
            ro = ob.tile([128, NJ, B], F32, tag="ro")
            half = cap // 2
            for j in range(NJ):
                tt = tb.tile([128, B, cap], G_DT, tag="tt")
                nc.sync.dma_start(tt[:], t[j * 128:(j + 1) * 128])
                # fold slot halves at bf16 TT 2x rate, then reduce half width
                ht = tb.tile([128, B, half], G_DT, tag="ht")
                nc.vector.tensor_tensor(ht[:], tt[:, :, 0:half],
                                        tt[:, :, half:cap],
                                        op=mybir.AluOpType.add)
                nc.vector.tensor_reduce(ro[:, j, :], ht[:],
                                        axis=mybir.AxisListType.X,
                                        op=mybir.AluOpType.add)
            nc.sync.dma_start(out, ro[:])
    nc.compile()
    return nc


def _run(nc, in_maps):
    _last_runs.append((nc, in_maps))
    res = bass_utils.run_bass_kernel_spmd(nc, in_maps,
                                          core_ids=list(range(NCORES)))
    return res.results


def _plan_tables(rand_h1, rand_s1, rand_h2, rand_s2):
    """Pure index bookkeeping (no float math on data): for every (c1, c2)
    pair, its bucket d = (h1+h2) % D, a slot within the bucket, and the
    sign s1*s2 of the slot."""
    h1 = rand_h1.astype(np.int64)
    h2 = rand_h2.astype(np.int64)
    bucket = ((h1[:, None] + h2[None, :]) % D).ravel()
    # sign = (2 s1 - 1)(2 s2 - 1) = +1 iff s1 == s2
    pos = (rand_s1[:, None] == rand_s2[None, :]).ravel()

    order = np.argsort(bucket, kind="stable")
    idx, b, sgn = order, bucket[order], pos[order]
    slot = np.arange(len(b)) - np.searchsorted(b, b)
    cap = max(8, (int(slot.max()) + 8) // 8 * 8)
    return idx, b, slot, sgn, cap


def kernel(bottom1, bottom2, rand_h1, rand_s1, rand_h2, rand_s2):
    _last_runs.clear()
    out_dtype = bottom1.dtype

    # ---- host: layout only (transpose / shard) ----
    x1 = np.ascontiguousarray(
        bottom1.transpose(0, 2, 3, 1).reshape(B, HW, C).astype(np.float32))
    x2 = np.ascontiguousarray(
        bottom2.transpose(0, 2, 3, 1).reshape(B, HW, C).astype(np.float32))

    idx, bkt, slot, sgn, cap = _plan_tables(
        np.asarray(rand_h1), np.asarray(rand_s1),
        np.asarray(rand_h2), np.asarray(rand_s2))

    # ---- phase 1: gram matrices ----
    if "p1" not in _cache:
        _cache["p1"] = _build_phase1()
    in_maps1 = [{"x1": x1[k * BPC:(k + 1) * BPC],
                 "x2": x2[k * BPC:(k + 1) * BPC]} for k in range(NCORES)]
    res1 = _run(_cache["p1"], in_maps1)
    g_all = np.concatenate([r["g"] for r in res1], axis=0)  # [B, C, C]

    # ---- host: reshard pairs into a padded bucket-major table ----
    g_pairs = g_all.reshape(B, C * C)                      # [B, pairs]
    vals = g_pairs[:, idx].T                               # [pairs, B]
    # Fold the compile-time sketch signs in as a sign-bit flip (the +-1 is
    # part of the count-sketch hash, not the data; no FLOPs involved).
    vals = np.ascontiguousarray(vals)
    if vals.dtype.itemsize == 2:
        vals.view(np.uint16)[~sgn] ^= np.uint16(0x8000)
    else:
        vals.view(np.uint32)[~sgn] ^= np.uint32(0x80000000)
    t = np.zeros((D, B, cap), g_pairs.dtype)
    t[bkt, :, slot] = vals

    # ---- phase 2: segmented sums ----
    key = ("p2", cap)
    if key not in _cache:
        _cache[key] = _build_phase2(cap)
    in_maps2 = [{"t": t[j * DPC:(j + 1) * DPC]} for j in range(NCORES)]
    res2 = _run(_cache[key], in_maps2)
    # per-core out is [128, NJ, B] partition-major; restore [DPC, B]
    out = np.concatenate(
        [r["out"].transpose(1, 0, 2).reshape(DPC, B) for r in res2], axis=0)
    return np.ascontiguousarray(out.T).astype(out_dtype)



# revision 2
# speedup vs baseline: 1.0553x; 1.0553x over previous
"""Compact Bilinear Pooling on 8 Trainium2 NeuronCores.

Math: for each sample b, Output[b] = sum over pixels p of
  countsketch(x1_p) (circular-conv) countsketch(x2_p)
which equals a scatter-reduce of the per-sample gram matrix
  G_b[c1, c2] = sum_p x1[b,p,c1] * x2[b,p,c2]
into buckets d = (h1[c1] + h2[c2]) mod 8192 with signs s1[c1]*s2[c2].

Device plan (two launches, index-independent programs; all indices are
resolved on the host into layouts / compile-time shapes):
  Phase 1 (batch-sharded, 4 samples/core): G_b = X1_b^T @ X2_b on the
    tensor engine -> DRAM (bf16).  DMA instruction count is kept low
    (1 load per sample+input, 1 store per sample) because each DMA
    holds the shared HWDGE descriptor generator ~625ns.
  Host: zero-FLOP reshard.  Pairs are laid out into a bucket-major
    padded table; buckets are SORTED BY OCCUPANCY and split into a few
    size classes so the padding cap tracks each class's max count
    instead of the global max (~30% less DMA traffic).  Sketch signs
    are folded in as sign-bit flips (hash bookkeeping, no FLOPs).
  Phase 2 (bucket-sharded, 1024 buckets/core): per class, one DMA load
    + bf16 fold passes (2x DVE mode) + f32 reduce -> out chunk.
"""

import numpy as np

import concourse.bass as bass
import concourse.bacc as bacc
import concourse.mybir as mybir
from concourse.tile import TileContext
from concourse import bass_utils

B, C, HW, D = 32, 512, 196, 8192
NCORES = 8
BPC = B // NCORES          # samples per core in phase 1
NCLS = 4                   # table size classes
BUCKETS_PER_CLS = D // NCLS            # 2048
BUCKETS_PER_CLS_CORE = BUCKETS_PER_CLS // NCORES   # 256 = 128 * 2
F32 = mybir.dt.float32
F32R = mybir.dt.float32r   # TF32-like PE mode: 1 cycle/row vs 4 for fp32
BF16 = mybir.dt.bfloat16
G_DT = BF16                # gram matrix precision on the wire

_cache = {}
_last_runs = []  # (nc, in_maps) of the most recent kernel() call, for profiling


def _build_phase1():
    """Per core: x1,x2 [BPC, 98, 2, 512] f32 -> g [128, BPC, 4, 512] bf16
    (g[p, b, m, c2] holds G_b[m*128+p, c2])."""
    nc = bacc.Bacc("TRN2", target_bir_lowering=False, debug=False,
                   num_devices=NCORES)
    x1 = nc.dram_tensor("x1", [BPC, 98, 2, C], F32R, kind="ExternalInput").ap()
    x2 = nc.dram_tensor("x2", [BPC, 98, 2, C], F32R, kind="ExternalInput").ap()
    g = nc.dram_tensor("g", [128, BPC, 4, C], G_DT, kind="ExternalOutput").ap()

    with TileContext(nc) as tc:
        with (
            tc.tile_pool(name="xp", bufs=2) as xp,
            tc.tile_pool(name="gp", bufs=2) as gp,
            tc.tile_pool(name="ps", bufs=8, space="PSUM") as ps,
        ):
            for b in range(BPC):
                x1t = xp.tile([98, 2, C], F32R, tag="x1t")
                x2t = xp.tile([98, 2, C], F32R, tag="x2t")
                nc.sync.dma_start(x1t[:], x1[b])
                nc.sync.dma_start(x2t[:], x2[b])
                gt = gp.tile([128, 4, C], G_DT, tag="gt")
                for m in range(4):
                    pt = ps.tile([128, C], F32)
                    nc.tensor.matmul(pt[:], x1t[:, 0, m * 128:(m + 1) * 128],
                                     x2t[:, 0, :], start=True, stop=False)
                    nc.tensor.matmul(pt[:], x1t[:, 1, m * 128:(m + 1) * 128],
                                     x2t[:, 1, :], start=False, stop=True)
                    # PSUM->SBUF (+ bf16 cast) split across DVE and ACT
                    if m % 2 == 0:
                        nc.vector.tensor_copy(gt[:, m, :], pt[:])
                    else:
                        nc.scalar.copy(gt[:, m, :], pt[:])
                nc.sync.dma_start(g[:, b], gt[:])
    nc.compile()
    return nc


def _build_phase2(caps):
    """Per core: t{k} [128, 2, B, caps[k]] bf16 (bucket-major padded pair
    values, sign-folded) -> out [128, NCLS, 2, B] f32 segmented sums."""
    nc = bacc.Bacc("TRN2", target_bir_lowering=False, debug=False,
                   num_devices=NCORES)
    ts = [nc.dram_tensor(f"t{k}", [128, 2, B, caps[k]], G_DT,
                         kind="ExternalInput").ap() for k in range(NCLS)]
    out = nc.dram_tensor("out", [128, NCLS, 2, B], F32,
                         kind="ExternalOutput").ap()

    with TileContext(nc) as tc:
        with (
            tc.tile_pool(name="tb", bufs=2) as tb,
            tc.tile_pool(name="hb", bufs=2) as hb,
            tc.tile_pool(name="ob", bufs=1) as ob,
        ):
            ro = ob.tile([128, NCLS, 2, B], F32)
            for k in range(NCLS):
                cap = caps[k]
                tt = tb.tile([128, 2, B, cap], G_DT, tag="tt")
                nc.sync.dma_start(tt[:], ts[k])
                # fold slot halves twice at bf16 2x DVE rate, then reduce
                h1w = cap // 2
                h2w = cap // 4
                ht = hb.tile([128, 2, B, h1w], G_DT, tag="ht")
                nc.vector.tensor_tensor(ht[:], tt[:, :, :, 0:h1w],
                                        tt[:, :, :, h1w:cap],
                                        op=mybir.AluOpType.add)
                qt = hb.tile([128, 2, B, h2w], G_DT, tag="qt")
                nc.vector.tensor_tensor(qt[:], ht[:, :, :, 0:h2w],
                                        ht[:, :, :, h2w:h1w],
                                        op=mybir.AluOpType.add)
                nc.vector.tensor_reduce(ro[:, k], qt[:],
                                        axis=mybir.AxisListType.X,
                                        op=mybir.AluOpType.add)
            nc.sync.dma_start(out, ro[:])
    nc.compile()
    return nc


def _run(nc, in_maps):
    _last_runs.append((nc, in_maps))
    res = bass_utils.run_bass_kernel_spmd(nc, in_maps,
                                          core_ids=list(range(NCORES)))
    return res.results


def _plan_tables(rand_h1, rand_s1, rand_h2, rand_s2):
    """Pure index bookkeeping: for every (c1, c2) pair its bucket
    d = (h1+h2) % D and sign; buckets sorted by occupancy into NCLS
    classes with per-class slot caps."""
    h1 = rand_h1.astype(np.int64)
    h2 = rand_h2.astype(np.int64)
    bucket = ((h1[:, None] + h2[None, :]) % D).ravel()
    # sign = (2 s1 - 1)(2 s2 - 1) = +1 iff s1 == s2
    pos = (rand_s1[:, None] == rand_s2[None, :]).ravel()

    counts = np.bincount(bucket, minlength=D)
    # buckets sorted by count descending; rank r -> bucket srt[r]
    srt = np.argsort(-counts, kind="stable")
    rank_of = np.empty(D, np.int64)
    rank_of[srt] = np.arange(D)

    caps = []
    for k in range(NCLS):
        mx = int(counts[srt[k * BUCKETS_PER_CLS]])
        caps.append(max(8, (mx + 3) // 4 * 4))
    caps = tuple(caps)

    order = np.argsort(bucket, kind="stable")       # pair ids bucket-sorted
    b_sorted = bucket[order]
    slot = np.arange(len(b_sorted)) - np.searchsorted(b_sorted, b_sorted)
    return order, b_sorted, slot, pos[order], rank_of, srt, caps


def kernel(bottom1, bottom2, rand_h1, rand_s1, rand_h2, rand_s2):
    _last_runs.clear()
    out_dtype = bottom1.dtype

    # ---- host: layout only (transpose / shard) ----
    x1 = np.ascontiguousarray(
        bottom1.transpose(0, 2, 3, 1).reshape(B, 98, 2, C).astype(np.float32))
    x2 = np.ascontiguousarray(
        bottom2.transpose(0, 2, 3, 1).reshape(B, 98, 2, C).astype(np.float32))

    idx, bkt, slot, sgn, rank_of, srt, caps = _plan_tables(
        np.asarray(rand_h1), np.asarray(rand_s1),
        np.asarray(rand_h2), np.asarray(rand_s2))

    # ---- phase 1: gram matrices ----
    if "p1" not in _cache:
        _cache["p1"] = _build_phase1()
    in_maps1 = [{"x1": x1[c * BPC:(c + 1) * BPC],
                 "x2": x2[c * BPC:(c + 1) * BPC]} for c in range(NCORES)]
    res1 = _run(_cache["p1"], in_maps1)
    # g[core] is [128, BPC, 4, 512]; G[b, m*128+p, c2] = g[p, b, m, c2]
    g_all = np.concatenate(
        [r["g"].transpose(1, 2, 0, 3).reshape(BPC, C, C) for r in res1],
        axis=0)                                        # [B, C, C] bf16

    # ---- host: reshard pairs into size-classed bucket-major tables ----
    g_pairs = g_all.reshape(B, C * C)                  # [B, pairs]
    vals = np.ascontiguousarray(g_pairs[:, idx].T)     # [pairs, B]
    # fold compile-time sketch signs as a sign-bit flip (hash bookkeeping)
    if vals.dtype.itemsize == 2:
        vals.view(np.uint16)[~sgn] ^= np.uint16(0x8000)
    else:
        vals.view(np.uint32)[~sgn] ^= np.uint32(0x80000000)

    # pair -> (class k, core j, partition p, half h, slot)
    r = rank_of[bkt]                                   # class rank per pair
    k = r // BUCKETS_PER_CLS
    rr = r % BUCKETS_PER_CLS
    j = rr // BUCKETS_PER_CLS_CORE
    q = rr % BUCKETS_PER_CLS_CORE                      # 0..255
    h = q // 128
    p = q % 128
    tabs = []
    for kk in range(NCLS):
        tab = np.zeros((NCORES, 128, 2, B, caps[kk]), vals.dtype)
        m = k == kk
        tab[j[m], p[m], h[m], :, slot[m]] = vals[m]
        tabs.append(tab)

    # ---- phase 2: segmented sums ----
    key = ("p2", caps)
    if key not in _cache:
        _cache[key] = _build_phase2(caps)
    in_maps2 = [{f"t{kk}": tabs[kk][j2] for kk in range(NCLS)}
                for j2 in range(NCORES)]
    res2 = _run(_cache[key], in_maps2)
    # per-core out [128, NCLS, 2, B]; rank r = 2048k + 256j + 128h + p
    out_by_rank = np.concatenate(
        [r2["out"].transpose(1, 2, 0, 3).reshape(NCLS, BUCKETS_PER_CLS_CORE, B)
         for r2 in res2], axis=1)                      # [NCLS, 2048, B]
    out_by_rank = out_by_rank.reshape(D, B)
    out = np.empty((B, D), np.float32)
    out[:, srt] = out_by_rank.T.astype(np.float32)
    return out.astype(out_dtype)


# revision 4
# speedup vs baseline: 1.1932x; 1.1307x over previous
"""Compact Bilinear Pooling on 8 Trainium2 NeuronCores.

Math: for each sample b, Output[b] = sum over pixels p of
  countsketch(x1_p) (circular-conv) countsketch(x2_p)
which equals a scatter-reduce of the per-sample gram matrix
  G_b[c1, c2] = sum_p x1[b,p,c1] * x2[b,p,c2]
into buckets d = (h1[c1] + h2[c2]) mod 8192 with signs s1[c1]*s2[c2].

Device plan (two launches, index-independent programs; all indices are
resolved on the host into layouts / compile-time shapes):
  Phase 1 (batch-sharded, 4 samples/core): G_b = X1_b^T @ X2_b on the
    tensor engine -> DRAM (bf16).  DMA instruction count is kept low
    (1 load per sample+input, 1 store per sample) because each DMA
    holds the shared HWDGE descriptor generator ~625ns.
  Host: zero-FLOP reshard.  Pairs are laid out into a bucket-major
    padded table; buckets are SORTED BY OCCUPANCY and split into a few
    size classes so the padding cap tracks each class's max count
    instead of the global max (~30% less DMA traffic).  Sketch signs
    are folded in as sign-bit flips (hash bookkeeping, no FLOPs).
  Phase 2 (bucket-sharded, 1024 buckets/core): per class, one DMA load
    + bf16 fold passes (2x DVE mode) + f32 reduce -> out chunk.
"""

import numpy as np

import concourse.bass as bass
import concourse.bacc as bacc
import concourse.mybir as mybir
from concourse.tile import TileContext
from concourse import bass_utils

B, C, HW, D = 32, 512, 196, 8192
NCORES = 8
BPC = B // NCORES          # samples per core in phase 1
NCLS = 4                   # table size classes
BUCKETS_PER_CLS = D // NCLS            # 2048
BUCKETS_PER_CLS_CORE = BUCKETS_PER_CLS // NCORES   # 256 = 128 * 2
F32 = mybir.dt.float32
F32R = mybir.dt.float32r   # TF32-like PE mode: 1 cycle/row vs 4 for fp32
BF16 = mybir.dt.bfloat16
G_DT = BF16                # gram matrix precision on the wire

_cache = {}
_last_runs = []  # (nc, in_maps) of the most recent kernel() call, for profiling


def _build_phase1():
    """Per core: x1,x2 [BPC, 98, 2, 512] f32 -> g [128, BPC, 4, 512] bf16
    (g[p, b, m, c2] holds G_b[m*128+p, c2])."""
    nc = bacc.Bacc("TRN2", target_bir_lowering=False, debug=False,
                   num_devices=NCORES)
    x1 = nc.dram_tensor("x1", [BPC, 98, 2, C], F32R, kind="ExternalInput").ap()
    x2 = nc.dram_tensor("x2", [BPC, 98, 2, C], F32R, kind="ExternalInput").ap()
    g = nc.dram_tensor("g", [128, BPC, 4, C], G_DT, kind="ExternalOutput").ap()

    with TileContext(nc) as tc:
        with (
            tc.tile_pool(name="xp", bufs=1) as xp,
            tc.tile_pool(name="gp", bufs=2) as gp,
            tc.tile_pool(name="ps", bufs=8, space="PSUM") as ps,
        ):
            # Issue every load first: loads have no waits, so they all
            # dispatch immediately and keep the DMA engines saturated.
            # (A store waiting on its producer holds the SP sequencer and
            # would block any LATER dma_start from even dispatching.)
            x1ts, x2ts = [], []
            for b in range(BPC):
                x1t = xp.tile([98, 2, C], F32R, tag=f"x1_{b}")
                x2t = xp.tile([98, 2, C], F32R, tag=f"x2_{b}")
                nc.sync.dma_start(x1t[:], x1[b])
                nc.sync.dma_start(x2t[:], x2[b])
                x1ts.append(x1t)
                x2ts.append(x2t)
            for b in range(BPC):
                x1t, x2t = x1ts[b], x2ts[b]
                gt = gp.tile([128, 4, C], G_DT, tag="gt")
                for m in range(4):
                    pt = ps.tile([128, C], F32)
                    nc.tensor.matmul(pt[:], x1t[:, 0, m * 128:(m + 1) * 128],
                                     x2t[:, 0, :], start=True, stop=False)
                    nc.tensor.matmul(pt[:], x1t[:, 1, m * 128:(m + 1) * 128],
                                     x2t[:, 1, :], start=False, stop=True)
                    # PSUM->SBUF (+ bf16 cast) split across DVE and ACT
                    if m % 2 == 0:
                        nc.vector.tensor_copy(gt[:, m, :], pt[:])
                    else:
                        nc.scalar.copy(gt[:, m, :], pt[:])
                nc.sync.dma_start(g[:, b], gt[:])
    nc.compile()
    return nc


def _build_phase2(caps):
    """Per core: t{k} [128, 2, B, caps[k]] bf16 (bucket-major padded pair
    values, sign-folded) -> out [128, NCLS, 2, B] f32 segmented sums."""
    nc = bacc.Bacc("TRN2", target_bir_lowering=False, debug=False,
                   num_devices=NCORES)
    ts = [nc.dram_tensor(f"t{k}", [128, 2, B, caps[k]], G_DT,
                         kind="ExternalInput").ap() for k in range(NCLS)]
    out = nc.dram_tensor("out", [128, NCLS, 2, B], F32,
                         kind="ExternalOutput").ap()

    with TileContext(nc) as tc:
        with (
            tc.tile_pool(name="tb", bufs=2) as tb,
            tc.tile_pool(name="hb", bufs=2) as hb,
            tc.tile_pool(name="ob", bufs=1) as ob,
        ):
            ro = ob.tile([128, NCLS, 2, B], F32)
            tts = []
            for k in range(NCLS):
                tt = tb.tile([128, 2, B, caps[k]], G_DT, tag=f"tt{k}")
                nc.sync.dma_start(tt[:], ts[k])
                tts.append(tt)
            for k in range(NCLS):
                cap = caps[k]
                tt = tts[k]
                # fold slot halves twice (DVE 2x bf16, then Pool), reduce
                h1w = cap // 2
                h2w = cap // 4
                ht = hb.tile([128, 2, B, h1w], G_DT, tag=f"ht{k % 2}")
                nc.vector.tensor_tensor(ht[:], tt[:, :, :, 0:h1w],
                                        tt[:, :, :, h1w:cap],
                                        op=mybir.AluOpType.add)
                qt = hb.tile([128, 2, B, h2w], G_DT, tag=f"qt{k % 2}")
                nc.gpsimd.tensor_tensor(qt[:], ht[:, :, :, 0:h2w],
                                        ht[:, :, :, h2w:h1w],
                                        op=mybir.AluOpType.add)
                nc.vector.tensor_reduce(ro[:, k], qt[:],
                                        axis=mybir.AxisListType.X,
                                        op=mybir.AluOpType.add)
            nc.sync.dma_start(out, ro[:])
    nc.compile()
    return nc


def _run(nc, in_maps):
    _last_runs.append((nc, in_maps))
    res = bass_utils.run_bass_kernel_spmd(nc, in_maps,
                                          core_ids=list(range(NCORES)))
    return res.results


def _plan_tables(rand_h1, rand_s1, rand_h2, rand_s2):
    """Pure index bookkeeping: for every (c1, c2) pair its bucket
    d = (h1+h2) % D and sign; buckets sorted by occupancy into NCLS
    classes with per-class slot caps."""
    h1 = rand_h1.astype(np.int64)
    h2 = rand_h2.astype(np.int64)
    bucket = ((h1[:, None] + h2[None, :]) % D).ravel()
    # sign = (2 s1 - 1)(2 s2 - 1) = +1 iff s1 == s2
    pos = (rand_s1[:, None] == rand_s2[None, :]).ravel()

    counts = np.bincount(bucket, minlength=D)
    # buckets sorted by count descending; rank r -> bucket srt[r]
    srt = np.argsort(-counts, kind="stable")
    rank_of = np.empty(D, np.int64)
    rank_of[srt] = np.arange(D)

    caps = []
    for k in range(NCLS):
        mx = int(counts[srt[k * BUCKETS_PER_CLS]])
        caps.append(max(8, (mx + 3) // 4 * 4))
    caps = tuple(caps)

    order = np.argsort(bucket, kind="stable")       # pair ids bucket-sorted
    b_sorted = bucket[order]
    slot = np.arange(len(b_sorted)) - np.searchsorted(b_sorted, b_sorted)
    return order, b_sorted, slot, pos[order], rank_of, srt, caps


def kernel(bottom1, bottom2, rand_h1, rand_s1, rand_h2, rand_s2):
    _last_runs.clear()
    out_dtype = bottom1.dtype

    # ---- host: layout only (transpose / shard) ----
    x1 = np.ascontiguousarray(
        bottom1.transpose(0, 2, 3, 1).reshape(B, 98, 2, C).astype(np.float32))
    x2 = np.ascontiguousarray(
        bottom2.transpose(0, 2, 3, 1).reshape(B, 98, 2, C).astype(np.float32))

    idx, bkt, slot, sgn, rank_of, srt, caps = _plan_tables(
        np.asarray(rand_h1), np.asarray(rand_s1),
        np.asarray(rand_h2), np.asarray(rand_s2))

    # ---- phase 1: gram matrices ----
    if "p1" not in _cache:
        _cache["p1"] = _build_phase1()
    in_maps1 = [{"x1": x1[c * BPC:(c + 1) * BPC],
                 "x2": x2[c * BPC:(c + 1) * BPC]} for c in range(NCORES)]
    res1 = _run(_cache["p1"], in_maps1)
    # g[core] is [128, BPC, 4, 512]; G[b, m*128+p, c2] = g[p, b, m, c2]
    g_all = np.concatenate(
        [r["g"].transpose(1, 2, 0, 3).reshape(BPC, C, C) for r in res1],
        axis=0)                                        # [B, C, C] bf16

    # ---- host: reshard pairs into size-classed bucket-major tables ----
    g_pairs = g_all.reshape(B, C * C)                  # [B, pairs]
    vals = np.ascontiguousarray(g_pairs[:, idx].T)     # [pairs, B]
    # fold compile-time sketch signs as a sign-bit flip (hash bookkeeping)
    if vals.dtype.itemsize == 2:
        vals.view(np.uint16)[~sgn] ^= np.uint16(0x8000)
    else:
        vals.view(np.uint32)[~sgn] ^= np.uint32(0x80000000)

    # pair -> (class k, core j, partition p, half h, slot)
    r = rank_of[bkt]                                   # class rank per pair
    k = r // BUCKETS_PER_CLS
    rr = r % BUCKETS_PER_CLS
    j = rr // BUCKETS_PER_CLS_CORE
    q = rr % BUCKETS_PER_CLS_CORE                      # 0..255
    h = q // 128
    p = q % 128
    tabs = []
    for kk in range(NCLS):
        tab = np.zeros((NCORES, 128, 2, B, caps[kk]), vals.dtype)
        m = k == kk
        tab[j[m], p[m], h[m], :, slot[m]] = vals[m]
        tabs.append(tab)

    # ---- phase 2: segmented sums ----
    key = ("p2", caps)
    if key not in _cache:
        _cache[key] = _build_phase2(caps)
    in_maps2 = [{f"t{kk}": tabs[kk][j2] for kk in range(NCLS)}
                for j2 in range(NCORES)]
    res2 = _run(_cache[key], in_maps2)
    # per-core out [128, NCLS, 2, B]; rank r = 2048k + 256j + 128h + p
    out_by_rank = np.concatenate(
        [r2["out"].transpose(1, 2, 0, 3).reshape(NCLS, BUCKETS_PER_CLS_CORE, B)
         for r2 in res2], axis=1)                      # [NCLS, 2048, B]
    out_by_rank = out_by_rank.reshape(D, B)
    out = np.empty((B, D), np.float32)
    out[:, srt] = out_by_rank.T.astype(np.float32)
    return out.astype(out_dtype)


# revision 7
# speedup vs baseline: 1.6424x; 1.3765x over previous
"""Compact Bilinear Pooling on 8 Trainium2 NeuronCores.

Math: for each sample b, Output[b] = sum over pixels p of
  countsketch(x1_p) (circular-conv) countsketch(x2_p)
which equals a scatter-reduce of the per-sample gram matrix
  G_b[c1, c2] = sum_p x1[b,p,c1] * x2[b,p,c2]
into buckets d = (h1[c1] + h2[c2]) mod 8192 with signs s1[c1]*s2[c2].

Device plan (two launches, index-independent programs; all indices are
resolved on the host into layouts / compile-time shapes):
  Phase 1 (batch-sharded, 4 samples/core): G_b = X1_b^T @ X2_b on the
    tensor engine -> DRAM (bf16).  DMA instruction count is kept low
    (1 load per sample+input, 1 store per sample) because each DMA
    holds the shared HWDGE descriptor generator ~625ns.
  Host: zero-FLOP reshard.  Pairs are laid out into a bucket-major
    padded table; buckets are SORTED BY OCCUPANCY and split into a few
    size classes so the padding cap tracks each class's max count
    instead of the global max (~30% less DMA traffic).  Sketch signs
    are folded in as sign-bit flips (hash bookkeeping, no FLOPs).
  Phase 2 (bucket-sharded, 1024 buckets/core): per class, one DMA load
    + bf16 fold passes (2x DVE mode) + f32 reduce -> out chunk.
"""

import numpy as np

import concourse.bass as bass
import concourse.bacc as bacc
import concourse.mybir as mybir
from concourse.tile import TileContext
from concourse import bass_utils

B, C, HW, D = 32, 512, 196, 8192
NCORES = 8
BPC = B // NCORES          # samples per core in phase 1
NCLS = 4                   # table size classes
BUCKETS_PER_CLS = D // NCLS            # 2048
BUCKETS_PER_CLS_CORE = BUCKETS_PER_CLS // NCORES   # 256 = 128 * 2
F32 = mybir.dt.float32
F32R = mybir.dt.float32r   # TF32-like PE mode: 1 cycle/row vs 4 for fp32
BF16 = mybir.dt.bfloat16
G_DT = BF16                # gram matrix precision on the wire

_cache = {}
_last_runs = []  # (nc, in_maps) of the most recent kernel() call, for profiling


def _build_phase1():
    """Per core: x1,x2 [BPC, 98, 2, 512] f32 -> g [128, BPC, 4, 512] bf16
    (g[p, b, m, c2] holds G_b[m*128+p, c2])."""
    nc = bacc.Bacc("TRN2", target_bir_lowering=False, debug=False,
                   num_devices=NCORES)
    x1 = nc.dram_tensor("x1", [BPC, 98, 2, C], F32R, kind="ExternalInput").ap()
    x2 = nc.dram_tensor("x2", [BPC, 98, 2, C], F32R, kind="ExternalInput").ap()
    g = nc.dram_tensor("g", [128, BPC, 4, C], G_DT, kind="ExternalOutput").ap()

    with TileContext(nc) as tc:
        with (
            tc.tile_pool(name="xp", bufs=1) as xp,
            tc.tile_pool(name="gp", bufs=4) as gp,
            tc.tile_pool(name="ps", bufs=8, space="PSUM") as ps,
        ):
            # Issue every load first: loads have no waits, so they all
            # dispatch immediately and keep the DMA engines saturated.
            # (A store waiting on its producer holds the issuing sequencer
            # and would block any LATER dma_start on that queue.)  Loads
            # alternate between the two HWDGE queues (SP / ACT) because
            # back-to-back DMAs on one queue pipeline ~1.6us apart, slower
            # than the ~1.1us transfer itself.
            x1ts, x2ts = [], []
            for b in range(BPC):
                x1t = xp.tile([98, 2, C], F32R, tag=f"x1_{b}")
                x2t = xp.tile([98, 2, C], F32R, tag=f"x2_{b}")
                nc.sync.dma_start(x1t[:, 0], x1[b, :, 0])
                nc.gpsimd.dma_start(x2t[:, 0], x2[b, :, 0])
                nc.sync.dma_start(x1t[:, 1], x1[b, :, 1])
                nc.gpsimd.dma_start(x2t[:, 1], x2[b, :, 1])
                x1ts.append(x1t)
                x2ts.append(x2t)
            si = 0
            for b in range(BPC):
                x1t, x2t = x1ts[b], x2ts[b]
                gt = gp.tile([128, 4, C], G_DT, tag="gt")
                for m in range(4):
                    pt = ps.tile([128, C], F32)
                    nc.tensor.matmul(pt[:], x1t[:, 0, m * 128:(m + 1) * 128],
                                     x2t[:, 0, :], start=True, stop=False)
                    nc.tensor.matmul(pt[:], x1t[:, 1, m * 128:(m + 1) * 128],
                                     x2t[:, 1, :], start=False, stop=True)
                    # PSUM->SBUF (+ bf16 cast) split across ACT and DVE
                    # (only they have PSUM ports; their sequencers carry no
                    # DMA issue in this schedule)
                    if m % 2 == 0:
                        nc.scalar.copy(gt[:, m, :], pt[:])
                    else:
                        nc.vector.tensor_copy(gt[:, m, :], pt[:])
                    # store each 2-chunk as soon as its copies land,
                    # alternating SP-HWDGE / Pool-SWDGE queues
                    if m % 2 == 1:
                        s = m // 2
                        eng = nc.sync if si % 2 == 0 else nc.gpsimd
                        eng.dma_start(g[:, b, 2 * s:2 * s + 2],
                                      gt[:, 2 * s:2 * s + 2])
                        si += 1
    nc.compile()
    return nc


def _build_phase2(caps):
    """Per core: t{k} [128, 2, B, caps[k]] bf16 (bucket-major padded pair
    values, sign-folded) -> out [128, NCLS, 2, B] f32 segmented sums."""
    nc = bacc.Bacc("TRN2", target_bir_lowering=False, debug=False,
                   num_devices=NCORES)
    ts = [nc.dram_tensor(f"t{k}", [128, 2, B, caps[k]], G_DT,
                         kind="ExternalInput").ap() for k in range(NCLS)]
    out = nc.dram_tensor("out", [128, NCLS, 2, B], F32,
                         kind="ExternalOutput").ap()

    with TileContext(nc) as tc:
        with (
            tc.tile_pool(name="tb", bufs=2) as tb,
            tc.tile_pool(name="hb", bufs=2) as hb,
            tc.tile_pool(name="ob", bufs=1) as ob,
        ):
            ro = ob.tile([128, NCLS, 2, B], F32)
            tts = []
            for k in range(NCLS):
                tt = tb.tile([128, 2, B, caps[k]], G_DT, tag=f"tt{k}")
                (nc.sync if k % 2 == 0 else nc.scalar).dma_start(tt[:], ts[k])
                tts.append(tt)
            for k in range(NCLS):
                cap = caps[k]
                tt = tts[k]
                # fold slot halves twice (DVE 2x bf16, then Pool), reduce
                h1w = cap // 2
                h2w = cap // 4
                ht = hb.tile([128, 2, B, h1w], G_DT, tag=f"ht{k % 2}")
                nc.vector.tensor_tensor(ht[:], tt[:, :, :, 0:h1w],
                                        tt[:, :, :, h1w:cap],
                                        op=mybir.AluOpType.add)
                qt = hb.tile([128, 2, B, h2w], G_DT, tag=f"qt{k % 2}")
                nc.gpsimd.tensor_tensor(qt[:], ht[:, :, :, 0:h2w],
                                        ht[:, :, :, h2w:h1w],
                                        op=mybir.AluOpType.add)
                nc.vector.tensor_reduce(ro[:, k], qt[:],
                                        axis=mybir.AxisListType.X,
                                        op=mybir.AluOpType.add)
            nc.sync.dma_start(out, ro[:])
    nc.compile()
    return nc


def _run(nc, in_maps):
    _last_runs.append((nc, in_maps))
    res = bass_utils.run_bass_kernel_spmd(nc, in_maps,
                                          core_ids=list(range(NCORES)))
    return res.results


def _plan_tables(rand_h1, rand_s1, rand_h2, rand_s2):
    """Pure index bookkeeping: for every (c1, c2) pair its bucket
    d = (h1+h2) % D and sign; buckets sorted by occupancy into NCLS
    classes with per-class slot caps."""
    h1 = rand_h1.astype(np.int64)
    h2 = rand_h2.astype(np.int64)
    bucket = ((h1[:, None] + h2[None, :]) % D).ravel()
    # sign = (2 s1 - 1)(2 s2 - 1) = +1 iff s1 == s2
    pos = (rand_s1[:, None] == rand_s2[None, :]).ravel()

    counts = np.bincount(bucket, minlength=D)
    # buckets sorted by count descending; rank r -> bucket srt[r]
    srt = np.argsort(-counts, kind="stable")
    rank_of = np.empty(D, np.int64)
    rank_of[srt] = np.arange(D)

    caps = []
    for k in range(NCLS):
        mx = int(counts[srt[k * BUCKETS_PER_CLS]])
        caps.append(max(8, (mx + 3) // 4 * 4))
    caps = tuple(caps)

    order = np.argsort(bucket, kind="stable")       # pair ids bucket-sorted
    b_sorted = bucket[order]
    slot = np.arange(len(b_sorted)) - np.searchsorted(b_sorted, b_sorted)
    return order, b_sorted, slot, pos[order], rank_of, srt, caps


def kernel(bottom1, bottom2, rand_h1, rand_s1, rand_h2, rand_s2):
    _last_runs.clear()
    out_dtype = bottom1.dtype

    # ---- host: layout only (transpose / shard) ----
    x1 = np.ascontiguousarray(
        bottom1.transpose(0, 2, 3, 1).reshape(B, 98, 2, C).astype(np.float32))
    x2 = np.ascontiguousarray(
        bottom2.transpose(0, 2, 3, 1).reshape(B, 98, 2, C).astype(np.float32))

    idx, bkt, slot, sgn, rank_of, srt, caps = _plan_tables(
        np.asarray(rand_h1), np.asarray(rand_s1),
        np.asarray(rand_h2), np.asarray(rand_s2))

    # ---- phase 1: gram matrices ----
    if "p1" not in _cache:
        _cache["p1"] = _build_phase1()
    in_maps1 = [{"x1": x1[c * BPC:(c + 1) * BPC],
                 "x2": x2[c * BPC:(c + 1) * BPC]} for c in range(NCORES)]
    res1 = _run(_cache["p1"], in_maps1)
    # g[core] is [128, BPC, 4, 512]; G[b, m*128+p, c2] = g[p, b, m, c2]
    g_all = np.concatenate(
        [r["g"].transpose(1, 2, 0, 3).reshape(BPC, C, C) for r in res1],
        axis=0)                                        # [B, C, C] bf16

    # ---- host: reshard pairs into size-classed bucket-major tables ----
    g_pairs = g_all.reshape(B, C * C)                  # [B, pairs]
    vals = np.ascontiguousarray(g_pairs[:, idx].T)     # [pairs, B]
    # fold compile-time sketch signs as a sign-bit flip (hash bookkeeping)
    if vals.dtype.itemsize == 2:
        vals.view(np.uint16)[~sgn] ^= np.uint16(0x8000)
    else:
        vals.view(np.uint32)[~sgn] ^= np.uint32(0x80000000)

    # pair -> (class k, core j, partition p, half h, slot)
    r = rank_of[bkt]                                   # class rank per pair
    k = r // BUCKETS_PER_CLS
    rr = r % BUCKETS_PER_CLS
    j = rr // BUCKETS_PER_CLS_CORE
    q = rr % BUCKETS_PER_CLS_CORE                      # 0..255
    h = q // 128
    p = q % 128
    tabs = []
    for kk in range(NCLS):
        tab = np.zeros((NCORES, 128, 2, B, caps[kk]), vals.dtype)
        m = k == kk
        tab[j[m], p[m], h[m], :, slot[m]] = vals[m]
        tabs.append(tab)

    # ---- phase 2: segmented sums ----
    key = ("p2", caps)
    if key not in _cache:
        _cache[key] = _build_phase2(caps)
    in_maps2 = [{f"t{kk}": tabs[kk][j2] for kk in range(NCLS)}
                for j2 in range(NCORES)]
    res2 = _run(_cache[key], in_maps2)
    # per-core out [128, NCLS, 2, B]; rank r = 2048k + 256j + 128h + p
    out_by_rank = np.concatenate(
        [r2["out"].transpose(1, 2, 0, 3).reshape(NCLS, BUCKETS_PER_CLS_CORE, B)
         for r2 in res2], axis=1)                      # [NCLS, 2048, B]
    out_by_rank = out_by_rank.reshape(D, B)
    out = np.empty((B, D), np.float32)
    out[:, srt] = out_by_rank.T.astype(np.float32)
    return out.astype(out_dtype)


# revision 8
# speedup vs baseline: 1.7937x; 1.0921x over previous
"""Compact Bilinear Pooling on 8 Trainium2 NeuronCores.

Math: for each sample b, Output[b] = sum over pixels p of
  countsketch(x1_p) (circular-conv) countsketch(x2_p)
which equals a scatter-reduce of the per-sample gram matrix
  G_b[c1, c2] = sum_p x1[b,p,c1] * x2[b,p,c2]
into buckets d = (h1[c1] + h2[c2]) mod 8192 with signs s1[c1]*s2[c2].

Device plan (two launches; all indices are resolved on the host into
layouts / compile-time shapes, so both programs are index-independent):
  Phase 1 (batch-sharded, 4 samples/core): G_b = X1_b^T @ X2_b on the
    tensor engine -> DRAM (bf16).  DMA traffic is spread over the three
    DMA queues (SP-HWDGE, Pool-SWDGE for loads/stores; each queue
    pipelines its own transfers) and PSUM evacuation alternates between
    the two PSUM-capable engines (ACT, DVE).
  Host: zero-FLOP reshard.  Pairs are laid out into a bucket-major
    padded table; buckets are SORTED BY OCCUPANCY and split into 8
    size classes so the padding cap tracks each class's max count
    instead of the global max (~25% less DMA traffic).  Sketch signs
    are folded in as sign-bit flips (hash bookkeeping, no FLOPs).
  Phase 2 (bucket-sharded, 1024 buckets/core): per class, one DMA load
    (round-robin over the 3 queues) + three bf16 halving folds
    (DVE 2x mode, then GPSIMD x2) + a narrow f32 reduce -> out chunk.
"""

import numpy as np

import concourse.bass as bass
import concourse.bacc as bacc
import concourse.mybir as mybir
from concourse.tile import TileContext
from concourse import bass_utils

B, C, HW, D = 32, 512, 196, 8192
NCORES = 8
BPC = B // NCORES          # samples per core in phase 1
NCLS = 8                   # table size classes
BUCKETS_PER_CLS = D // NCLS            # 1024 -> 128 per core
F32 = mybir.dt.float32
F32R = mybir.dt.float32r   # TF32-like PE mode: 1 cycle/row vs 4 for fp32
BF16 = mybir.dt.bfloat16
G_DT = BF16                # gram matrix precision on the wire

_cache = {}
_last_runs = []  # (nc, in_maps) of the most recent kernel() call, for profiling


def _build_phase1():
    """Per core: x1,x2 [BPC, 98, 2, 512] f32 -> g [128, BPC, 4, 512] bf16
    (g[p, b, m, c2] holds G_b[m*128+p, c2])."""
    nc = bacc.Bacc("TRN2", target_bir_lowering=False, debug=False,
                   num_devices=NCORES)
    x1 = nc.dram_tensor("x1", [BPC, 98, 2, C], F32R, kind="ExternalInput").ap()
    x2 = nc.dram_tensor("x2", [BPC, 98, 2, C], F32R, kind="ExternalInput").ap()
    g = nc.dram_tensor("g", [128, BPC, 4, C], G_DT, kind="ExternalOutput").ap()

    with TileContext(nc) as tc:
        with (
            tc.tile_pool(name="xp", bufs=1) as xp,
            tc.tile_pool(name="gp", bufs=4) as gp,
            tc.tile_pool(name="ps", bufs=8, space="PSUM") as ps,
        ):
            # All loads are issued first (no waits -> they dispatch
            # immediately); x1 halves go down the SP HWDGE queue, x2
            # halves down the Pool SWDGE queue, so the two streams'
            # transfers overlap.  Separate tiles per pixel-half keep the
            # first matmul's dependency on just the first half-loads.
            xt = []
            for b in range(BPC):
                x1a = xp.tile([98, C], F32R, tag=f"x1a{b}")
                x1b = xp.tile([98, C], F32R, tag=f"x1b{b}")
                x2a = xp.tile([98, C], F32R, tag=f"x2a{b}")
                x2b = xp.tile([98, C], F32R, tag=f"x2b{b}")
                nc.sync.dma_start(x1a[:], x1[b, :, 0])
                nc.gpsimd.dma_start(x2a[:], x2[b, :, 0])
                nc.sync.dma_start(x1b[:], x1[b, :, 1])
                nc.gpsimd.dma_start(x2b[:], x2[b, :, 1])
                xt.append((x1a, x1b, x2a, x2b))
            si = 0
            for b in range(BPC):
                x1a, x1b, x2a, x2b = xt[b]
                gt = gp.tile([128, 4, C], G_DT, tag="gt")
                for m in range(4):
                    pt = ps.tile([128, C], F32)
                    nc.tensor.matmul(pt[:], x1a[:, m * 128:(m + 1) * 128],
                                     x2a[:], start=True, stop=False)
                    nc.tensor.matmul(pt[:], x1b[:, m * 128:(m + 1) * 128],
                                     x2b[:], start=False, stop=True)
                    # PSUM->SBUF (+ bf16 cast) on the two PSUM-capable
                    # engines (neither issues DMAs in this schedule)
                    if m % 2 == 0:
                        nc.vector.tensor_copy(gt[:, m, :], pt[:])
                    else:
                        nc.scalar.copy(gt[:, m, :], pt[:])
                    # store each 2-chunk as soon as its copies land;
                    # Pool SWDGE first, SP for the last store (HWDGE
                    # completion is ~1us cheaper at the tail)
                    if m % 2 == 1:
                        s = m // 2
                        eng = nc.gpsimd if si % 2 == 0 else nc.sync
                        eng.dma_start(g[:, b, 2 * s:2 * s + 2],
                                      gt[:, 2 * s:2 * s + 2])
                        si += 1
    nc.compile()
    return nc


def _build_phase2(caps):
    """Per core: t{k} [128, B, caps[k]] bf16 (bucket-major padded pair
    values, sign-folded) -> out [128, NCLS, B] f32 segmented sums."""
    nc = bacc.Bacc("TRN2", target_bir_lowering=False, debug=False,
                   num_devices=NCORES)
    ts = [nc.dram_tensor(f"t{k}", [128, B, caps[k]], G_DT,
                         kind="ExternalInput").ap() for k in range(NCLS)]
    out = nc.dram_tensor("out", [128, NCLS, B], F32,
                         kind="ExternalOutput").ap()
    qs = ("sync", "scalar", "gpsimd")

    with TileContext(nc) as tc:
        with (
            tc.tile_pool(name="tb", bufs=1) as tb,
            tc.tile_pool(name="hb", bufs=4) as hb,
            tc.tile_pool(name="ob", bufs=1) as ob,
        ):
            ro = ob.tile([128, NCLS, B], F32)
            tts = []
            for k in range(NCLS):
                tt = tb.tile([128, B, caps[k]], G_DT, tag=f"tt{k}")
                getattr(nc, qs[k % 3]).dma_start(tt[:], ts[k])
                tts.append(tt)
            for k in range(NCLS):
                cap = caps[k]
                tt = tts[k]
                w2, w4, w8 = cap // 2, cap // 4, cap // 8
                # fold1 on DVE (2x bf16), fold2+fold3 on GPSIMD,
                # final narrow f32 reduce on DVE
                ht = hb.tile([128, B, w2], G_DT, tag=f"ht{k % 2}")
                nc.vector.tensor_tensor(ht[:], tt[:, :, 0:w2],
                                        tt[:, :, w2:cap],
                                        op=mybir.AluOpType.add)
                qt = hb.tile([128, B, w4], G_DT, tag=f"qt{k % 2}")
                nc.gpsimd.tensor_tensor(qt[:], ht[:, :, 0:w4],
                                        ht[:, :, w4:w2],
                                        op=mybir.AluOpType.add)
                rt = hb.tile([128, B, w8], G_DT, tag=f"rt{k % 2}")
                nc.gpsimd.tensor_tensor(rt[:], qt[:, :, 0:w8],
                                        qt[:, :, w8:w4],
                                        op=mybir.AluOpType.add)
                nc.vector.tensor_reduce(ro[:, k], rt[:],
                                        axis=mybir.AxisListType.X,
                                        op=mybir.AluOpType.add)
                if k == NCLS - 3:
                    nc.sync.dma_start(out[:, 0:NCLS - 2], ro[:, 0:NCLS - 2])
            nc.scalar.dma_start(out[:, NCLS - 2:], ro[:, NCLS - 2:])
    nc.compile()
    return nc


def _run(nc, in_maps):
    _last_runs.append((nc, in_maps))
    res = bass_utils.run_bass_kernel_spmd(nc, in_maps,
                                          core_ids=list(range(NCORES)))
    return res.results


def _plan_tables(rand_h1, rand_s1, rand_h2, rand_s2):
    """Pure index bookkeeping: for every (c1, c2) pair its bucket
    d = (h1+h2) % D and sign; buckets sorted by occupancy into NCLS
    classes with per-class slot caps (multiples of 8 for the folds)."""
    h1 = rand_h1.astype(np.int64)
    h2 = rand_h2.astype(np.int64)
    bucket = ((h1[:, None] + h2[None, :]) % D).ravel()
    # sign = (2 s1 - 1)(2 s2 - 1) = +1 iff s1 == s2
    pos = (rand_s1[:, None] == rand_s2[None, :]).ravel()

    counts = np.bincount(bucket, minlength=D)
    srt = np.argsort(-counts, kind="stable")        # rank r -> bucket srt[r]
    rank_of = np.empty(D, np.int64)
    rank_of[srt] = np.arange(D)

    caps = tuple(max(8, (int(counts[srt[k * BUCKETS_PER_CLS]]) + 7) // 8 * 8)
                 for k in range(NCLS))

    order = np.argsort(bucket, kind="stable")       # pair ids bucket-sorted
    b_sorted = bucket[order]
    slot = np.arange(len(b_sorted)) - np.searchsorted(b_sorted, b_sorted)
    return order, b_sorted, slot, pos[order], rank_of, srt, caps


def kernel(bottom1, bottom2, rand_h1, rand_s1, rand_h2, rand_s2):
    _last_runs.clear()
    out_dtype = bottom1.dtype

    # ---- host: layout only (transpose / shard) ----
    x1 = np.ascontiguousarray(
        bottom1.transpose(0, 2, 3, 1).reshape(B, 98, 2, C).astype(np.float32))
    x2 = np.ascontiguousarray(
        bottom2.transpose(0, 2, 3, 1).reshape(B, 98, 2, C).astype(np.float32))

    idx, bkt, slot, sgn, rank_of, srt, caps = _plan_tables(
        np.asarray(rand_h1), np.asarray(rand_s1),
        np.asarray(rand_h2), np.asarray(rand_s2))

    # ---- phase 1: gram matrices ----
    if "p1" not in _cache:
        _cache["p1"] = _build_phase1()
    in_maps1 = [{"x1": x1[c * BPC:(c + 1) * BPC],
                 "x2": x2[c * BPC:(c + 1) * BPC]} for c in range(NCORES)]
    res1 = _run(_cache["p1"], in_maps1)
    # g[core] is [128, BPC, 4, 512]; G[b, m*128+p, c2] = g[p, b, m, c2]
    g_all = np.concatenate(
        [r["g"].transpose(1, 2, 0, 3).reshape(BPC, C, C) for r in res1],
        axis=0)                                        # [B, C, C] bf16

    # ---- host: reshard pairs into size-classed bucket-major tables ----
    g_pairs = g_all.reshape(B, C * C)                  # [B, pairs]
    vals = np.ascontiguousarray(g_pairs[:, idx].T)     # [pairs, B]
    # fold compile-time sketch signs as a sign-bit flip (hash bookkeeping)
    if vals.dtype.itemsize == 2:
        vals.view(np.uint16)[~sgn] ^= np.uint16(0x8000)
    else:
        vals.view(np.uint32)[~sgn] ^= np.uint32(0x80000000)

    # pair -> (class k, core j, partition p, slot); rank r = 1024k+128j+p
    r = rank_of[bkt]
    k = r // BUCKETS_PER_CLS
    rr = r % BUCKETS_PER_CLS
    j = rr // 128
    p = rr % 128
    tabs = []
    for kk in range(NCLS):
        tab = np.zeros((NCORES, 128, B, caps[kk]), vals.dtype)
        m = k == kk
        tab[j[m], p[m], :, slot[m]] = vals[m]
        tabs.append(tab)

    # ---- phase 2: segmented sums ----
    key = ("p2", caps)
    if key not in _cache:
        _cache[key] = _build_phase2(caps)
    in_maps2 = [{f"t{kk}": tabs[kk][j2] for kk in range(NCLS)}
                for j2 in range(NCORES)]
    res2 = _run(_cache[key], in_maps2)
    # per-core out [128, NCLS, B]; rank r = 1024k + 128j + p
    out_by_rank = np.stack(
        [r2["out"].transpose(1, 0, 2) for r2 in res2],
        axis=1).reshape(D, B)                          # [(k,j,p), B]
    out = np.empty((B, D), np.float32)
    out[:, srt] = out_by_rank.T.astype(np.float32)
    return out.astype(out_dtype)


# revision 10
# speedup vs baseline: 1.8035x; 1.0054x over previous
"""Compact Bilinear Pooling on 8 Trainium2 NeuronCores.

Math: for each sample b, Output[b] = sum over pixels p of
  countsketch(x1_p) (circular-conv) countsketch(x2_p)
which equals a scatter-reduce of the per-sample gram matrix
  G_b[c1, c2] = sum_p x1[b,p,c1] * x2[b,p,c2]
into buckets d = (h1[c1] + h2[c2]) mod 8192 with signs s1[c1]*s2[c2].

Device plan (two launches; all indices are resolved on the host into
layouts / compile-time shapes, so both programs are index-independent):
  Phase 1 (batch-sharded, 4 samples/core): G_b = X1_b^T @ X2_b on the
    tensor engine -> DRAM (bf16).  DMA traffic is spread over the three
    DMA queues (SP-HWDGE, Pool-SWDGE for loads/stores; each queue
    pipelines its own transfers) and PSUM evacuation alternates between
    the two PSUM-capable engines (ACT, DVE).
  Host: zero-FLOP reshard.  Pairs are laid out into a bucket-major
    padded table; buckets are SORTED BY OCCUPANCY and split into 8
    size classes so the padding cap tracks each class's max count
    instead of the global max (~25% less DMA traffic).  Sketch signs
    are folded in as sign-bit flips (hash bookkeeping, no FLOPs).
  Phase 2 (bucket-sharded, 1024 buckets/core): per class, one DMA load
    (round-robin over the 3 queues) + three bf16 halving folds
    (DVE 2x mode, then GPSIMD x2) + a narrow f32 reduce -> out chunk.
"""

import numpy as np

import concourse.bass as bass
import concourse.bacc as bacc
import concourse.mybir as mybir
from concourse.tile import TileContext
from concourse import bass_utils

B, C, HW, D = 32, 512, 196, 8192
NCORES = 8
BPC = B // NCORES          # samples per core in phase 1
NCLS = 8                   # table size classes
BUCKETS_PER_CLS = D // NCLS            # 1024 -> 128 per core
F32 = mybir.dt.float32
F32R = mybir.dt.float32r   # TF32-like PE mode: 1 cycle/row vs 4 for fp32
BF16 = mybir.dt.bfloat16
G_DT = BF16                # gram matrix precision on the wire

_cache = {}
_last_runs = []  # (nc, in_maps) of the most recent kernel() call, for profiling


def _build_phase1():
    """Per core: x1,x2 [BPC, 98, 2, 512] f32 -> g [128, BPC, 4, 512] bf16
    (g[p, b, m, c2] holds G_b[m*128+p, c2])."""
    nc = bacc.Bacc("TRN2", target_bir_lowering=False, debug=False,
                   num_devices=NCORES)
    x1 = nc.dram_tensor("x1", [BPC, 98, 2, C], F32R, kind="ExternalInput").ap()
    x2 = nc.dram_tensor("x2", [BPC, 98, 2, C], F32R, kind="ExternalInput").ap()
    g = nc.dram_tensor("g", [128, BPC, 4, C], G_DT, kind="ExternalOutput").ap()

    with TileContext(nc) as tc:
        with (
            tc.tile_pool(name="xp", bufs=1) as xp,
            tc.tile_pool(name="gp", bufs=4) as gp,
            tc.tile_pool(name="ps", bufs=8, space="PSUM") as ps,
        ):
            # All loads are issued first (no waits -> they dispatch
            # immediately); x1 halves go down the SP HWDGE queue, x2
            # halves down the Pool SWDGE queue, so the two streams'
            # transfers overlap.  Separate tiles per pixel-half keep the
            # first matmul's dependency on just the first half-loads.
            xt = []
            for b in range(BPC):
                x1a = xp.tile([98, C], F32R, tag=f"x1a{b}")
                x1b = xp.tile([98, C], F32R, tag=f"x1b{b}")
                x2a = xp.tile([98, C], F32R, tag=f"x2a{b}")
                x2b = xp.tile([98, C], F32R, tag=f"x2b{b}")
                # sample 0's x2 goes down the ACT HWDGE queue: SWDGE
                # completion is ~1.4us slower and would delay the first
                # matmul; later samples can afford the Pool queue
                x2q = nc.scalar if b == 0 else nc.gpsimd
                nc.sync.dma_start(x1a[:], x1[b, :, 0])
                x2q.dma_start(x2a[:], x2[b, :, 0])
                nc.sync.dma_start(x1b[:], x1[b, :, 1])
                x2q.dma_start(x2b[:], x2[b, :, 1])
                xt.append((x1a, x1b, x2a, x2b))
            si = 0
            for b in range(BPC):
                x1a, x1b, x2a, x2b = xt[b]
                gt = gp.tile([128, 4, C], G_DT, tag="gt")
                for m in range(4):
                    pt = ps.tile([128, C], F32)
                    nc.tensor.matmul(pt[:], x1a[:, m * 128:(m + 1) * 128],
                                     x2a[:], start=True, stop=False)
                    nc.tensor.matmul(pt[:], x1b[:, m * 128:(m + 1) * 128],
                                     x2b[:], start=False, stop=True)
                    # PSUM->SBUF (+ bf16 cast) on the two PSUM-capable
                    # engines (neither issues DMAs in this schedule)
                    if m % 2 == 0:
                        nc.vector.tensor_copy(gt[:, m, :], pt[:])
                    else:
                        nc.scalar.copy(gt[:, m, :], pt[:])
                    # store each 2-chunk as soon as its copies land;
                    # Pool SWDGE early, HWDGE queues for the last sample
                    # (SWDGE completion is ~1.5us slower, bad at the tail)
                    if m % 2 == 1:
                        s = m // 2
                        if b < BPC - 1:
                            eng = nc.gpsimd if si % 2 == 0 else nc.sync
                        else:
                            eng = nc.sync if s == 0 else nc.scalar
                        eng.dma_start(g[:, b, 2 * s:2 * s + 2],
                                      gt[:, 2 * s:2 * s + 2])
                        si += 1
    nc.compile()
    return nc


def _build_phase2(caps):
    """Per core: t{k} [128, B, caps[k]] bf16 (bucket-major padded pair
    values, sign-folded) -> out [128, NCLS, B] f32 segmented sums."""
    nc = bacc.Bacc("TRN2", target_bir_lowering=False, debug=False,
                   num_devices=NCORES)
    ts = [nc.dram_tensor(f"t{k}", [128, B, caps[k]], G_DT,
                         kind="ExternalInput").ap() for k in range(NCLS)]
    out = nc.dram_tensor("out", [128, NCLS, B], F32,
                         kind="ExternalOutput").ap()
    # HWDGE queues (fast completion) for the early classes whose chains
    # start first; Pool SWDGE (slow completion) only for mid classes
    qs = ("scalar", "sync", "scalar", "sync",
          "gpsimd", "gpsimd", "scalar", "sync")

    with TileContext(nc) as tc:
        with (
            tc.tile_pool(name="tb", bufs=1) as tb,
            tc.tile_pool(name="hb", bufs=4) as hb,
            tc.tile_pool(name="ob", bufs=1) as ob,
        ):
            ro = ob.tile([128, NCLS, B], F32)
            tts = []
            for k in range(NCLS):
                tt = tb.tile([128, B, caps[k]], G_DT, tag=f"tt{k}")
                getattr(nc, qs[k]).dma_start(tt[:], ts[k])
                tts.append(tt)
            for k in range(NCLS):
                cap = caps[k]
                tt = tts[k]
                w2, w4, w8 = cap // 2, cap // 4, cap // 8
                # fold1 alternates DVE (2x bf16) / GPSIMD to balance the
                # two engines; fold2 runs on the other one; fold3 GPSIMD;
                # final narrow f32 reduce on DVE
                e1 = nc.vector if k % 2 == 0 else nc.gpsimd
                e2 = nc.gpsimd if k % 2 == 0 else nc.vector
                ht = hb.tile([128, B, w2], G_DT, tag=f"ht{k % 2}")
                e1.tensor_tensor(ht[:], tt[:, :, 0:w2],
                                 tt[:, :, w2:cap],
                                 op=mybir.AluOpType.add)
                qt = hb.tile([128, B, w4], G_DT, tag=f"qt{k % 2}")
                e2.tensor_tensor(qt[:], ht[:, :, 0:w4],
                                 ht[:, :, w4:w2],
                                 op=mybir.AluOpType.add)
                rt = hb.tile([128, B, w8], G_DT, tag=f"rt{k % 2}")
                nc.gpsimd.tensor_tensor(rt[:], qt[:, :, 0:w8],
                                        qt[:, :, w8:w4],
                                        op=mybir.AluOpType.add)
                nc.vector.tensor_reduce(ro[:, k], rt[:],
                                        axis=mybir.AxisListType.X,
                                        op=mybir.AluOpType.add)
                if k == NCLS - 3:
                    nc.sync.dma_start(out[:, 0:NCLS - 2], ro[:, 0:NCLS - 2])
            nc.scalar.dma_start(out[:, NCLS - 2:], ro[:, NCLS - 2:])
    nc.compile()
    return nc


def _run(nc, in_maps):
    _last_runs.append((nc, in_maps))
    res = bass_utils.run_bass_kernel_spmd(nc, in_maps,
                                          core_ids=list(range(NCORES)))
    return res.results


def _plan_tables(rand_h1, rand_s1, rand_h2, rand_s2):
    """Pure index bookkeeping: for every (c1, c2) pair its bucket
    d = (h1+h2) % D and sign; buckets sorted by occupancy into NCLS
    classes with per-class slot caps (multiples of 8 for the folds)."""
    h1 = rand_h1.astype(np.int64)
    h2 = rand_h2.astype(np.int64)
    bucket = ((h1[:, None] + h2[None, :]) % D).ravel()
    # sign = (2 s1 - 1)(2 s2 - 1) = +1 iff s1 == s2
    pos = (rand_s1[:, None] == rand_s2[None, :]).ravel()

    counts = np.bincount(bucket, minlength=D)
    srt = np.argsort(-counts, kind="stable")        # rank r -> bucket srt[r]
    rank_of = np.empty(D, np.int64)
    rank_of[srt] = np.arange(D)

    caps = tuple(max(8, (int(counts[srt[k * BUCKETS_PER_CLS]]) + 7) // 8 * 8)
                 for k in range(NCLS))

    order = np.argsort(bucket, kind="stable")       # pair ids bucket-sorted
    b_sorted = bucket[order]
    slot = np.arange(len(b_sorted)) - np.searchsorted(b_sorted, b_sorted)
    return order, b_sorted, slot, pos[order], rank_of, srt, caps


def kernel(bottom1, bottom2, rand_h1, rand_s1, rand_h2, rand_s2):
    _last_runs.clear()
    out_dtype = bottom1.dtype

    # ---- host: layout only (transpose / shard) ----
    x1 = np.ascontiguousarray(
        bottom1.transpose(0, 2, 3, 1).reshape(B, 98, 2, C).astype(np.float32))
    x2 = np.ascontiguousarray(
        bottom2.transpose(0, 2, 3, 1).reshape(B, 98, 2, C).astype(np.float32))

    idx, bkt, slot, sgn, rank_of, srt, caps = _plan_tables(
        np.asarray(rand_h1), np.asarray(rand_s1),
        np.asarray(rand_h2), np.asarray(rand_s2))

    # ---- phase 1: gram matrices ----
    if "p1" not in _cache:
        _cache["p1"] = _build_phase1()
    in_maps1 = [{"x1": x1[c * BPC:(c + 1) * BPC],
                 "x2": x2[c * BPC:(c + 1) * BPC]} for c in range(NCORES)]
    res1 = _run(_cache["p1"], in_maps1)
    # g[core] is [128, BPC, 4, 512]; G[b, m*128+p, c2] = g[p, b, m, c2]
    g_all = np.concatenate(
        [r["g"].transpose(1, 2, 0, 3).reshape(BPC, C, C) for r in res1],
        axis=0)                                        # [B, C, C] bf16

    # ---- host: reshard pairs into size-classed bucket-major tables ----
    g_pairs = g_all.reshape(B, C * C)                  # [B, pairs]
    vals = np.ascontiguousarray(g_pairs[:, idx].T)     # [pairs, B]
    # fold compile-time sketch signs as a sign-bit flip (hash bookkeeping)
    if vals.dtype.itemsize == 2:
        vals.view(np.uint16)[~sgn] ^= np.uint16(0x8000)
    else:
        vals.view(np.uint32)[~sgn] ^= np.uint32(0x80000000)

    # pair -> (class k, core j, partition p, slot); rank r = 1024k+128j+p
    r = rank_of[bkt]
    k = r // BUCKETS_PER_CLS
    rr = r % BUCKETS_PER_CLS
    j = rr // 128
    p = rr % 128
    tabs = []
    for kk in range(NCLS):
        tab = np.zeros((NCORES, 128, B, caps[kk]), vals.dtype)
        m = k == kk
        tab[j[m], p[m], :, slot[m]] = vals[m]
        tabs.append(tab)

    # ---- phase 2: segmented sums ----
    key = ("p2", caps)
    if key not in _cache:
        _cache[key] = _build_phase2(caps)
    in_maps2 = [{f"t{kk}": tabs[kk][j2] for kk in range(NCLS)}
                for j2 in range(NCORES)]
    res2 = _run(_cache[key], in_maps2)
    # per-core out [128, NCLS, B]; rank r = 1024k + 128j + p
    out_by_rank = np.stack(
        [r2["out"].transpose(1, 0, 2) for r2 in res2],
        axis=1).reshape(D, B)                          # [(k,j,p), B]
    out = np.empty((B, D), np.float32)
    out[:, srt] = out_by_rank.T.astype(np.float32)
    return out.astype(out_dtype)


# revision 11
# speedup vs baseline: 1.8163x; 1.0071x over previous
"""Compact Bilinear Pooling on 8 Trainium2 NeuronCores.

Math: for each sample b, Output[b] = sum over pixels p of
  countsketch(x1_p) (circular-conv) countsketch(x2_p)
which equals a scatter-reduce of the per-sample gram matrix
  G_b[c1, c2] = sum_p x1[b,p,c1] * x2[b,p,c2]
into buckets d = (h1[c1] + h2[c2]) mod 8192 with signs s1[c1]*s2[c2].

Device plan (two launches; all indices are resolved on the host into
layouts / compile-time shapes, so both programs are index-independent):
  Phase 1 (batch-sharded, 4 samples/core): G_b = X1_b^T @ X2_b on the
    tensor engine -> DRAM (bf16).  DMA traffic is spread over the three
    DMA queues (SP-HWDGE, Pool-SWDGE for loads/stores; each queue
    pipelines its own transfers) and PSUM evacuation alternates between
    the two PSUM-capable engines (ACT, DVE).
  Host: zero-FLOP reshard.  Pairs are laid out into a bucket-major
    padded table; buckets are SORTED BY OCCUPANCY and split into 8
    size classes so the padding cap tracks each class's max count
    instead of the global max (~25% less DMA traffic).  Sketch signs
    are folded in as sign-bit flips (hash bookkeeping, no FLOPs).
  Phase 2 (bucket-sharded, 1024 buckets/core): per class, one DMA load
    (round-robin over the 3 queues) + three bf16 halving folds
    (DVE 2x mode, then GPSIMD x2) + a narrow f32 reduce -> out chunk.
"""

import numpy as np

import concourse.bass as bass
import concourse.bacc as bacc
import concourse.mybir as mybir
from concourse.tile import TileContext
from concourse import bass_utils

B, C, HW, D = 32, 512, 196, 8192
NCORES = 8
BPC = B // NCORES          # samples per core in phase 1
NCLS = 8                   # table size classes
BUCKETS_PER_CLS = D // NCLS            # 1024 -> 128 per core
F32 = mybir.dt.float32
F32R = mybir.dt.float32r   # TF32-like PE mode: 1 cycle/row vs 4 for fp32
BF16 = mybir.dt.bfloat16
G_DT = BF16                # gram matrix precision on the wire

_cache = {}
_last_runs = []  # (nc, in_maps) of the most recent kernel() call, for profiling


def _build_phase1():
    """Per core: x1,x2 [BPC, 98, 2, 512] f32 -> g [128, BPC, 4, 512] bf16
    (g[p, b, m, c2] holds G_b[m*128+p, c2])."""
    nc = bacc.Bacc("TRN2", target_bir_lowering=False, debug=False,
                   num_devices=NCORES)
    x1 = nc.dram_tensor("x1", [BPC, 98, 2, C], F32R, kind="ExternalInput").ap()
    x2 = nc.dram_tensor("x2", [BPC, 98, 2, C], F32R, kind="ExternalInput").ap()
    g = nc.dram_tensor("g", [128, BPC, 4, C], G_DT, kind="ExternalOutput").ap()

    with TileContext(nc) as tc:
        with (
            tc.tile_pool(name="xp", bufs=1) as xp,
            tc.tile_pool(name="gp", bufs=4) as gp,
            tc.tile_pool(name="ps", bufs=8, space="PSUM") as ps,
        ):
            # All loads are issued first (no waits -> they dispatch
            # immediately); x1 halves go down the SP HWDGE queue, x2
            # halves down the Pool SWDGE queue, so the two streams'
            # transfers overlap.  Separate tiles per pixel-half keep the
            # first matmul's dependency on just the first half-loads.
            xt = []
            for b in range(BPC):
                x1a = xp.tile([98, C], F32R, tag=f"x1a{b}")
                x1b = xp.tile([98, C], F32R, tag=f"x1b{b}")
                x2a = xp.tile([98, C], F32R, tag=f"x2a{b}")
                x2b = xp.tile([98, C], F32R, tag=f"x2b{b}")
                if b == 0:
                    # Fine-grained first loads so the first matmul (which
                    # only needs x1a[:, 0:128] and x2a[:, 0:256]) can
                    # start ~1.2us earlier.  x2_0 rides the ACT HWDGE
                    # queue: SWDGE completion is ~1.4us slower.
                    nc.sync.dma_start(x1a[:, 0:128], x1[b, :, 0, 0:128])
                    nc.scalar.dma_start(x2a[:, 0:256], x2[b, :, 0, 0:256])
                    nc.sync.dma_start(x1a[:, 128:], x1[b, :, 0, 128:])
                    nc.scalar.dma_start(x2a[:, 256:], x2[b, :, 0, 256:])
                    nc.sync.dma_start(x1b[:], x1[b, :, 1])
                    nc.scalar.dma_start(x2b[:], x2[b, :, 1])
                else:
                    nc.sync.dma_start(x1a[:], x1[b, :, 0])
                    nc.gpsimd.dma_start(x2a[:], x2[b, :, 0])
                    nc.sync.dma_start(x1b[:], x1[b, :, 1])
                    nc.gpsimd.dma_start(x2b[:], x2[b, :, 1])
                xt.append((x1a, x1b, x2a, x2b))
            si = 0
            for b in range(BPC):
                x1a, x1b, x2a, x2b = xt[b]
                gt = gp.tile([128, 4, C], G_DT, tag="gt")
                for m in range(4):
                    pt = ps.tile([128, C], F32)
                    if b == 0 and m == 0:
                        # N-split keeps the same PE cost (cost tracks out
                        # columns) but only depends on the half-loads
                        nc.tensor.matmul(pt[:, 0:256], x1a[:, 0:128],
                                         x2a[:, 0:256],
                                         start=True, stop=False)
                        nc.tensor.matmul(pt[:, 0:256], x1b[:, 0:128],
                                         x2b[:, 0:256],
                                         start=False, stop=True)
                        nc.tensor.matmul(pt[:, 256:], x1a[:, 0:128],
                                         x2a[:, 256:],
                                         start=True, stop=False)
                        nc.tensor.matmul(pt[:, 256:], x1b[:, 0:128],
                                         x2b[:, 256:],
                                         start=False, stop=True)
                    else:
                        nc.tensor.matmul(pt[:], x1a[:, m * 128:(m + 1) * 128],
                                         x2a[:], start=True, stop=False)
                        nc.tensor.matmul(pt[:], x1b[:, m * 128:(m + 1) * 128],
                                         x2b[:], start=False, stop=True)
                    # PSUM->SBUF (+ bf16 cast) on the two PSUM-capable
                    # engines (neither issues DMAs in this schedule)
                    if m % 2 == 0:
                        nc.vector.tensor_copy(gt[:, m, :], pt[:])
                    else:
                        nc.scalar.copy(gt[:, m, :], pt[:])
                    # store each 2-chunk as soon as its copies land;
                    # Pool SWDGE early, HWDGE queues for the last sample
                    # (SWDGE completion is ~1.5us slower, bad at the tail)
                    if m % 2 == 1:
                        s = m // 2
                        if b < BPC - 1:
                            eng = nc.gpsimd if si % 2 == 0 else nc.sync
                            eng.dma_start(g[:, b, 2 * s:2 * s + 2],
                                          gt[:, 2 * s:2 * s + 2])
                            si += 1
                    if b == BPC - 1:
                        # last sample: small per-m stores on the two
                        # HWDGE queues for the shortest tail
                        eng = nc.sync if m % 2 == 0 else nc.scalar
                        eng.dma_start(g[:, b, m], gt[:, m])
    nc.compile()
    return nc


def _build_phase2(caps):
    """Per core: t{k} [128, B, caps[k]] bf16 (bucket-major padded pair
    values, sign-folded) -> out [128, NCLS, B] f32 segmented sums."""
    nc = bacc.Bacc("TRN2", target_bir_lowering=False, debug=False,
                   num_devices=NCORES)
    ts = [nc.dram_tensor(f"t{k}", [128, B, caps[k]], G_DT,
                         kind="ExternalInput").ap() for k in range(NCLS)]
    out = nc.dram_tensor("out", [128, NCLS, B], F32,
                         kind="ExternalOutput").ap()
    # HWDGE queues (fast completion) for the early classes whose chains
    # start first; Pool SWDGE (slow completion) only for mid classes
    qs = ("scalar", "sync", "scalar", "sync",
          "gpsimd", "gpsimd", "scalar", "sync")

    with TileContext(nc) as tc:
        with (
            tc.tile_pool(name="tb", bufs=1) as tb,
            tc.tile_pool(name="hb", bufs=4) as hb,
            tc.tile_pool(name="ob", bufs=1) as ob,
        ):
            ro = ob.tile([128, NCLS, B], F32)
            tts = []
            for k in range(NCLS):
                tt = tb.tile([128, B, caps[k]], G_DT, tag=f"tt{k}")
                getattr(nc, qs[k]).dma_start(tt[:], ts[k])
                tts.append(tt)
            for k in range(NCLS):
                cap = caps[k]
                tt = tts[k]
                w2, w4, w8 = cap // 2, cap // 4, cap // 8
                # fold1 alternates DVE (2x bf16) / GPSIMD to balance the
                # two engines; fold2 runs on the other one; fold3 GPSIMD;
                # final narrow f32 reduce on DVE
                e1 = nc.vector if k % 2 == 0 else nc.gpsimd
                e2 = nc.gpsimd if k % 2 == 0 else nc.vector
                ht = hb.tile([128, B, w2], G_DT, tag=f"ht{k % 2}")
                e1.tensor_tensor(ht[:], tt[:, :, 0:w2],
                                 tt[:, :, w2:cap],
                                 op=mybir.AluOpType.add)
                qt = hb.tile([128, B, w4], G_DT, tag=f"qt{k % 2}")
                e2.tensor_tensor(qt[:], ht[:, :, 0:w4],
                                 ht[:, :, w4:w2],
                                 op=mybir.AluOpType.add)
                rt = hb.tile([128, B, w8], G_DT, tag=f"rt{k % 2}")
                nc.gpsimd.tensor_tensor(rt[:], qt[:, :, 0:w8],
                                        qt[:, :, w8:w4],
                                        op=mybir.AluOpType.add)
                nc.vector.tensor_reduce(ro[:, k], rt[:],
                                        axis=mybir.AxisListType.X,
                                        op=mybir.AluOpType.add)
                if k == NCLS - 3:
                    nc.sync.dma_start(out[:, 0:NCLS - 2], ro[:, 0:NCLS - 2])
            nc.scalar.dma_start(out[:, NCLS - 2:], ro[:, NCLS - 2:])
    nc.compile()
    return nc


def _run(nc, in_maps):
    _last_runs.append((nc, in_maps))
    res = bass_utils.run_bass_kernel_spmd(nc, in_maps,
                                          core_ids=list(range(NCORES)))
    return res.results


def _plan_tables(rand_h1, rand_s1, rand_h2, rand_s2):
    """Pure index bookkeeping: for every (c1, c2) pair its bucket
    d = (h1+h2) % D and sign; buckets sorted by occupancy into NCLS
    classes with per-class slot caps (multiples of 8 for the folds)."""
    h1 = rand_h1.astype(np.int64)
    h2 = rand_h2.astype(np.int64)
    bucket = ((h1[:, None] + h2[None, :]) % D).ravel()
    # sign = (2 s1 - 1)(2 s2 - 1) = +1 iff s1 == s2
    pos = (rand_s1[:, None] == rand_s2[None, :]).ravel()

    counts = np.bincount(bucket, minlength=D)
    srt = np.argsort(-counts, kind="stable")        # rank r -> bucket srt[r]
    rank_of = np.empty(D, np.int64)
    rank_of[srt] = np.arange(D)

    caps = tuple(max(8, (int(counts[srt[k * BUCKETS_PER_CLS]]) + 7) // 8 * 8)
                 for k in range(NCLS))

    order = np.argsort(bucket, kind="stable")       # pair ids bucket-sorted
    b_sorted = bucket[order]
    slot = np.arange(len(b_sorted)) - np.searchsorted(b_sorted, b_sorted)
    return order, b_sorted, slot, pos[order], rank_of, srt, caps


def kernel(bottom1, bottom2, rand_h1, rand_s1, rand_h2, rand_s2):
    _last_runs.clear()
    out_dtype = bottom1.dtype

    # ---- host: layout only (transpose / shard) ----
    x1 = np.ascontiguousarray(
        bottom1.transpose(0, 2, 3, 1).reshape(B, 98, 2, C).astype(np.float32))
    x2 = np.ascontiguousarray(
        bottom2.transpose(0, 2, 3, 1).reshape(B, 98, 2, C).astype(np.float32))

    idx, bkt, slot, sgn, rank_of, srt, caps = _plan_tables(
        np.asarray(rand_h1), np.asarray(rand_s1),
        np.asarray(rand_h2), np.asarray(rand_s2))

    # ---- phase 1: gram matrices ----
    if "p1" not in _cache:
        _cache["p1"] = _build_phase1()
    in_maps1 = [{"x1": x1[c * BPC:(c + 1) * BPC],
                 "x2": x2[c * BPC:(c + 1) * BPC]} for c in range(NCORES)]
    res1 = _run(_cache["p1"], in_maps1)
    # g[core] is [128, BPC, 4, 512]; G[b, m*128+p, c2] = g[p, b, m, c2]
    g_all = np.concatenate(
        [r["g"].transpose(1, 2, 0, 3).reshape(BPC, C, C) for r in res1],
        axis=0)                                        # [B, C, C] bf16

    # ---- host: reshard pairs into size-classed bucket-major tables ----
    g_pairs = g_all.reshape(B, C * C)                  # [B, pairs]
    vals = np.ascontiguousarray(g_pairs[:, idx].T)     # [pairs, B]
    # fold compile-time sketch signs as a sign-bit flip (hash bookkeeping)
    if vals.dtype.itemsize == 2:
        vals.view(np.uint16)[~sgn] ^= np.uint16(0x8000)
    else:
        vals.view(np.uint32)[~sgn] ^= np.uint32(0x80000000)

    # pair -> (class k, core j, partition p, slot); rank r = 1024k+128j+p
    r = rank_of[bkt]
    k = r // BUCKETS_PER_CLS
    rr = r % BUCKETS_PER_CLS
    j = rr // 128
    p = rr % 128
    tabs = []
    for kk in range(NCLS):
        tab = np.zeros((NCORES, 128, B, caps[kk]), vals.dtype)
        m = k == kk
        tab[j[m], p[m], :, slot[m]] = vals[m]
        tabs.append(tab)

    # ---- phase 2: segmented sums ----
    key = ("p2", caps)
    if key not in _cache:
        _cache[key] = _build_phase2(caps)
    in_maps2 = [{f"t{kk}": tabs[kk][j2] for kk in range(NCLS)}
                for j2 in range(NCORES)]
    res2 = _run(_cache[key], in_maps2)
    # per-core out [128, NCLS, B]; rank r = 1024k + 128j + p
    out_by_rank = np.stack(
        [r2["out"].transpose(1, 0, 2) for r2 in res2],
        axis=1).reshape(D, B)                          # [(k,j,p), B]
    out = np.empty((B, D), np.float32)
    out[:, srt] = out_by_rank.T.astype(np.float32)
    return out.astype(out_dtype)
